# revision 47
# baseline (speedup 1.0000x reference)
"""BernNet (nn_BernNet_9543417332146) Trainium2 kernel.

Reference computation:
    h = relu(x @ W1 + b1) @ W2 + b2                      (MLP head)
    out = sum_j  C(K,j)/2^K * relu(temp)_j * L^j (2I-L)^{K-j} h
  with L = I - A  (A = sym-normalized adjacency), evaluated by the
  reference via 65 sparse matvecs.

All terms are polynomials in A and commute, so
    out = p(A) h,   p(l) = sum_j c_j T_j (1-l)^j (1+l)^{K-j}
a degree-K polynomial whose coefficients depend only on `temp`.  For
temp = ones (the initialized BernNet parameters), the binomial sum
telescopes:  sum_j C(K,j) (1-l)^j (1+l)^{K-j} = 2^K  =>  p == 1, i.e.
the whole graph propagation is the identity and out == h exactly.

This kernel computes the polynomial coefficients from `temp` at runtime
with exact integer arithmetic, runs the MLP on all 8 NeuronCores
(nodes row-sharded, weights replicated), and only performs sparse
matvec work for the (never-initialized) case of nonzero higher-degree
coefficients, via a Horner evaluation needing deg(p) matvecs instead of
the reference's 65.

Device program (16-bit IO; ~5-6 us/iteration/core vs the 14.3 us fp32
baseline):
  - inputs stream in bf16 (x pre-cast on host), output stored fp16 and
    upcast on host; rel tolerance is 2e-2, the 16-bit path lands ~3e-3.
    This halves both DMA directions (~781 KB of x in + ~781 KB of y out
    per core per iteration — weights are loaded once, outside the
    iteration loop — ~4.4 us at the ~360 GB/s per-core HBM share).
  - x packed transposed [128, HALF]: partitions 0..63 = features of the
    first HALF rows, 64..127 = features of the second HALF, so every DMA
    uses all 128 partitions contiguously.
  - mm1 runs the two 64-feature halves as row-tiled matmuls
    (tile_position rows 0/64, auto-derived from base partitions) into
    different PSUM banks (concurrent PE writes to one bank are a HW
    error); mm2 runs the two 64-feature output halves col-tiled into
    one bank (partitions 0-63 / 64-127), so every PSUM evacuation runs
    at the full 128-partition width.
  - the throughput bottleneck is PSUM evacuation: fp32 PSUM reads are
    1 elem/cycle/lane on both ACT (1.2 GHz) and DVE (0.96 GHz), 9408
    elems/partition/iteration => ~4.4 us floor. Evacuations are
    [128,1024] ops (4-slot PSUM pool) split across both engines with a
    balanced makespan; relu+b1 and the +b2/fp16 cast are fused into the
    evacuation ops.
  - loads are 3 pieces on the SP HWDGE ring; the single merged output
    store issues from the otherwise-idle GPSIMD (SWDGE), so the SDMA
    engines interleave loads/stores at packet granularity with no
    head-blocking and no sequencer time stolen from ACT/DVE.
"""

import numpy as np
from math import comb

N_NODES = 50000
FEATURES = 64
NHID = 128
NCORES = 8
ROWS_PER_CORE = 6250          # 8 * 6250 = 50000 exactly (no padding)
HALF = ROWS_PER_CORE // 2     # 3125 = 6*512 + 53
# per-half column chunks (start, width); <=512 so one fp32 PSUM bank each
CHUNKS = [(0, 512), (512, 512), (1024, 512), (1536, 512),
          (2048, 512), (2560, 512), (3072, 53)]
TAILW = CHUNKS[-1][1]
YGROUPS = [[0, 1], [2, 3], [4, 5], [6]]  # phase-2 (mm2+bias) psum groups

# Blob column layout (constants first so the first DMA piece covers them).
# b1/b2 are raw fp32 bits spanning 2 bf16 columns each (the elementwise
# engines need fp32 scalar operands; the device bitcasts them back — the
# runner's sim NaN-check is disabled since fp32 halfwords can alias bf16
# NaN patterns).
C_W1 = 0                      # [0,128)   W1 duplicated on both halves
C_B1 = NHID                   # [128,130) b1 per-partition (fp32 bits)
C_W2 = C_B1 + 2               # [130,194) W2 (all 128 partitions)
C_B2 = C_W2 + FEATURES        # [194,196) b2 duplicated (fp32 bits)
C_X = C_B2 + 2                # 196
BLOBW = C_X + HALF            # 3332

_nc_cache = {}


def _bern_poly_coefs(temp):
    """Coefficients a_m of p(A) = sum_m a_m A^m for the BernNet filter.

    p(l) = sum_j [C(K,j)/2^K] * relu(temp_j) * (1-l)^j (1+l)^{K-j}.
    The inner binomial products are exact integers, so for temp = ones
    the higher coefficients cancel to exactly 0.0 in float arithmetic.
    """
    k = temp.shape[0] - 1
    T = np.maximum(np.asarray(temp, np.float64), 0.0)
    a = np.zeros(k + 1)
    for j in range(k + 1):
        tj = T[j]
        if tj == 0.0:
            continue
        for m in range(k + 1):
            s = 0
            for p in range(max(0, m - (k - j)), min(j, m) + 1):
                s += (-1) ** p * comb(j, p) * comb(k - j, m - p)
            a[m] += (comb(k, j) * s) * tj / float(2**k)
    return a


def _build_mlp_nc(repeat=1, hw_loop=1):
    """SPMD per-core program: y = (relu(x@W1+b1))@W2+b2 for a 6250-row
    shard.  See module docstring for the dataflow.

    repeat: python-unrolled body repetitions (pipelined against each
    other).  hw_loop: wraps the unrolled body in a tc.For_i hardware
    loop (all-engine barrier per trip) — used by the test harness to put
    thousands of iterations in one NEFF so device time dominates the
    host dispatch overhead when measuring."""
    import concourse.bass as bass
    import concourse.bacc as bacc
    import concourse.mybir as mybir
    from concourse.tile import TileContext

    f32 = mybir.dt.float32
    bf16 = mybir.dt.bfloat16
    fp16 = mybir.dt.float16
    relu = mybir.ActivationFunctionType.Relu
    ident = mybir.ActivationFunctionType.Identity
    add_op = mybir.AluOpType.add
    max_op = mybir.AluOpType.max
    # Bacc (not bare Bass): its lowering legalizes multi-wait instructions
    # into fused event-semaphore sequences the TRN2 encoders accept.
    nc = bacc.Bacc(None, target_bir_lowering=False)

    blob = nc.dram_tensor("blob", [128, BLOBW], bf16, kind="ExternalInput")
    yt = nc.dram_tensor("yt", [128, HALF], fp16, kind="ExternalOutput")

    with TileContext(nc) as tc:
        with (
            tc.tile_pool(name="io", bufs=3) as iopool,
            tc.tile_pool(name="rt", bufs=8) as rtpool,
            tc.tile_pool(name="rt3", bufs=2) as rt3pool,
            tc.tile_pool(name="yc", bufs=2) as ycpool,
            tc.tile_pool(name="warmp", bufs=1) as warmpool,
            tc.tile_pool(name="psum", bufs=4, space=bass.MemorySpace.PSUM) as ppool,
        ):
            # Pre-warm the ACT function-table (LoadActFuncSet ~2.7us) and
            # the PE HAM clock before any data arrives.
            warm = warmpool.tile([1, 1], f32, tag="warm")
            nc.vector.memset(warm[:], 0.0)
            nc.scalar.activation(warm[:], warm[:], relu)
            scr = warmpool.tile([128, 384], bf16, tag="scr")
            nc.vector.memset(scr[:], 0.0)
            pw = ppool.tile([128, 1024], f32, tag="ps")
            for _ in range(3):
                nc.tensor.matmul(
                    pw[:, :256], scr[:64, :128], scr[:64, 128:384],
                    start=True, stop=True,
                )

            # Constants (W1 dup, biases, W2 — 196 cols) are loop-invariant:
            # load them once per NEFF execution into a persistent tile, so
            # the steady-state iteration DMA is x + y only.
            ct = warmpool.tile([128, C_X], bf16, tag="consts")
            nc.sync.dma_start(ct[:], blob[:, 0:C_X])
            w1h = [ct[0:64, C_W1:C_W1 + NHID],
                   ct[64:128, C_W1:C_W1 + NHID]]
            b1t = ct[:, C_B1:C_B1 + 2].bitcast(f32)
            w2t = ct[:, C_W2:C_W2 + FEATURES]
            b2t = ct[:, C_B2:C_B2 + 2].bitcast(f32)

            def body(_rep):
                bt = iopool.tile([128, HALF], bf16, tag="xtile")
                for p0c, p1c in ((0, 1024), (1024, 2048), (2048, HALF)):
                    nc.sync.dma_start(bt[:, p0c:p1c],
                                      blob[:, C_X + p0c:C_X + p1c])

                # Evacuation engine split (balanced makespan; ACT is a
                # bit faster per op so it carries one more big op).
                def evac_relu(act, dst, src):
                    if act:
                        nc.scalar.activation(dst, src, relu, bias=b1t)
                    else:
                        nc.vector.tensor_scalar(dst, src, b1t, 0.0,
                                                add_op, max_op)

                def evac_bias(act, dst, src):
                    if act:
                        nc.scalar.activation(dst, src, ident, bias=b2t)
                    else:
                        nc.vector.tensor_scalar_add(dst, src, b2t)

                rts = {}
                yc = ycpool.tile([128, HALF], fp16, tag="yc")

                def phase1(ci):
                    # mm1: the two row-tiled halves run concurrently on
                    # the PE, so they land in different PSUM banks
                    # (PE-W/PE-W same-bank collision is a hardware
                    # error), then relu(z+b1) fp32 PSUM -> bf16 SBUF.
                    c0, w = CHUNKS[ci]
                    ps = ppool.tile([128, 1024], f32, tag="ps")
                    for h in range(2):
                        # tail: h0 end-aligned at 512-w (bank 0), h1 at
                        # 512 (start of bank 1) — different banks for the
                        # concurrent PE writes, adjacent for the relu
                        pcol = h * 512 if ci < 6 else (512 - w) + h * w
                        xs = bt[64 * h:64 * h + 64, c0:c0 + w]
                        nc.tensor.matmul(ps[:, pcol:pcol + w],
                                         w1h[h], xs, start=True, stop=True)
                    if ci < 6:
                        rt = rtpool.tile([128, 1024], bf16, tag="rt")
                        evac_relu(ci % 2 == 0, rt[:, :1024], ps[:, :1024])
                    else:
                        # tail halves at 512-w/512: different banks for
                        # the concurrent PE writes, but adjacent columns
                        # so a single relu op evacuates both
                        rt = rt3pool.tile([128, 2 * TAILW], bf16, tag="rt3")
                        evac_relu(False, rt[:, 0:2 * TAILW],
                                  ps[:, 512 - TAILW:512 + TAILW])
                    rts[ci] = rt

                def phase2(yi):
                    # mm2: col-tiled h0/h1 pairs share a bank at
                    # partitions 0-63/64-127; then y+b2 -> fp16 yc slice.
                    ygroup = YGROUPS[yi]
                    ps2 = ppool.tile([128, 1024], f32, tag="ps")
                    for k, c in enumerate(ygroup):
                        c0, w = CHUNKS[c]
                        hoff = 512 if c < 6 else TAILW
                        for h in range(2):
                            mov = rts[c][:, h * hoff:h * hoff + w]
                            nc.tensor.matmul(
                                ps2[64 * h:64 * h + 64, k * 512:k * 512 + w],
                                w2t, mov, start=True, stop=True)
                    ybase = CHUNKS[ygroup[0]][0]
                    ew = sum(w for _, w in (CHUNKS[c] for c in ygroup))
                    evac_bias(yi != 1, yc[:, ybase:ybase + ew],
                              ps2[:, :ew])

                # Emission order: phase 1 fully, then phase 2 (each Y
                # group's matmuls sit well behind the relus that feed
                # them in PE program order, so they never head-block;
                # interleaving the phases measured strictly worse).
                for ci in range(7):
                    phase1(ci)
                for yi in range(4):
                    phase2(yi)
                # single output store from the otherwise-idle GPSIMD
                # (SWDGE), keeping the SP/ACT HWDGE rings for loads
                nc.gpsimd.dma_start(yt[:, :], yc[:, :])

            # repeat>1 re-runs the whole body (DMAs included) inside one
            # NEFF — the test harness measures steady-state HW time via
            # (T(R2)-T(R1))/(R2-R1), cancelling dispatch overhead.
            # hw_loop>1 additionally wraps the unrolled body in a
            # hardware loop so thousands of iterations run per dispatch.
            import contextlib
            loop_cm = (tc.For_i(0, hw_loop, 1) if hw_loop > 1
                       else contextlib.nullcontext())
            with loop_cm:
                for r in range(repeat):
                    body(r)
    nc.compile()
    return nc


def _build_blobs(x, W1, b1, W2, b2):
    """Full inputs -> bf16 blobs [NCORES, 128, BLOBW] (biases embedded as
    raw fp32 bits across 2 bf16 columns each)."""
    import ml_dtypes

    n = x.shape[0]
    n_pad = NCORES * ROWS_PER_CORE
    x_pad = np.zeros((n_pad, FEATURES), np.float32)
    x_pad[:n] = x
    consts = np.zeros((128, C_X), np.float32)
    consts[:, C_W1:C_W1 + NHID] = np.concatenate([W1, W1], axis=0)
    consts[:, C_W2:C_W2 + FEATURES] = W2

    blob_all = np.empty((NCORES, 128, BLOBW), np.float32)
    blob_all[:, :, :C_X] = consts
    blob_all[:, :, C_X:] = (
        x_pad.reshape(NCORES, 2, HALF, FEATURES)
        .transpose(0, 1, 3, 2)
        .reshape(NCORES, 128, HALF)
    )
    out = blob_all.astype(ml_dtypes.bfloat16)
    u16 = out.view(np.uint16)
    u16[:, :, C_B1:C_B1 + 2] = b1.astype(np.float32).view(np.uint16).reshape(128, 2)
    u16[:, :, C_B2:C_B2 + 2] = (
        np.concatenate([b2, b2]).astype(np.float32).view(np.uint16).reshape(128, 2)
    )
    return out


def _unpack_y(y_all, n=N_NODES):
    """[NCORES, 128, HALF] fp16 device output -> [n, 64] fp32 rows."""
    h = (
        np.asarray(y_all, np.float32)
        .reshape(NCORES, 2, FEATURES, HALF)
        .transpose(0, 1, 3, 2)
        .reshape(NCORES * ROWS_PER_CORE, FEATURES)
    )
    return np.ascontiguousarray(h[:n])


def _mlp_numpy(x, W1, b1, W2, b2):
    return np.maximum(x @ W1 + b1, 0.0) @ W2 + b2


def _make_runner(nc, n_cores=NCORES):
    """Persistent jitted executor for a prebuilt Bass module (mirrors
    bass2jax.run_bass_via_pjrt's sharded path, but jit-compiled once and
    without donation so it can be invoked repeatedly for timing).

    Returns (fn, in_names, out_names, out_avals): fn takes the
    axis-0-concatenated per-core inputs followed by concatenated zero
    output buffers and returns concatenated outputs.
    """
    import jax
    import concourse.mybir as mybir
    from concourse import bass2jax
    from jax.experimental.shard_map import shard_map
    from jax.sharding import Mesh, PartitionSpec

    bass2jax.install_neuronx_cc_hook()
    partition_name = nc.partition_id_tensor.name if nc.partition_id_tensor else None
    in_names, out_names, out_avals = [], [], []
    for alloc in nc.m.functions[0].allocations:
        if not isinstance(alloc, mybir.MemoryLocationSet):
            continue
        name = alloc.memorylocations[0].name
        if alloc.kind == "ExternalInput":
            if name != partition_name:
                in_names.append(name)
        elif alloc.kind == "ExternalOutput":
            out_names.append(name)
            out_avals.append(
                jax.core.ShapedArray(
                    tuple(alloc.tensor_shape), mybir.dt.np(alloc.dtype)
                )
            )
    n_params = len(in_names)
    all_in = list(in_names) + list(out_names)
    if partition_name is not None:
        all_in.append(partition_name)

    def _body(*args):
        operands = list(args)
        if partition_name is not None:
            operands.append(bass2jax.partition_id_tensor())
        return tuple(
            bass2jax._bass_exec_p.bind(
                *operands,
                out_avals=tuple(out_avals),
                in_names=tuple(all_in),
                out_names=tuple(out_names),
                lowering_input_output_aliases=(),
                # fp32 bias bits embedded in the bf16 blob can alias NaN
                # bf16 patterns — the value checks would false-positive
                sim_require_finite=False,
                sim_require_nnan=False,
                nc=nc,
            )
        )

    import numpy as _np

    devices = jax.devices()[:n_cores]
    mesh = Mesh(_np.asarray(devices), ("core",))
    nin = n_params + len(out_names)
    fn = jax.jit(
        shard_map(
            _body,
            mesh=mesh,
            in_specs=(PartitionSpec("core"),) * nin,
            out_specs=(PartitionSpec("core"),) * len(out_names),
            check_rep=False,
        ),
        keep_unused=True,
    )
    return fn, in_names, out_names, out_avals


def _mlp_trn(x, W1, b1, W2, b2, trace=False):
    """Run the MLP row-sharded across the 8 NeuronCores. Returns
    (h, exec_time_ns) — exec_time_ns is only populated when an NTFF
    profiling hook is available (trace=True); the test harness instead
    measures HW time via inner-repeat deltas.

    Uses a persistent jitted executable (cached across calls) so repeat
    kernel() invocations skip the XLA re-trace/re-compile that
    run_bass_kernel_spmd pays per call."""
    n = x.shape[0]
    if "nc" not in _nc_cache:
        _nc_cache["nc"] = _build_mlp_nc()
    nc = _nc_cache["nc"]

    if "runner" not in _nc_cache:
        _nc_cache["runner"] = _make_runner(nc)
    fn, in_names, out_names, out_avals = _nc_cache["runner"]
    assert in_names == ["blob"] and out_names == ["yt"]

    concat_blob = _build_blobs(x, W1, b1, W2, b2).reshape(NCORES * 128, BLOBW)
    zeros = np.zeros((NCORES * 128, HALF), np.float16)
    outs = fn(concat_blob, zeros)
    y = np.asarray(outs[0]).reshape(NCORES, 128, HALF)
    return _unpack_y(y, n), None


def kernel(x, edge_index, W1, b1, W2, b2, temp):
    x = np.asarray(x, np.float32)
    W1 = np.asarray(W1, np.float32)
    b1 = np.asarray(b1, np.float32)
    W2 = np.asarray(W2, np.float32)
    b2 = np.asarray(b2, np.float32)
    temp = np.asarray(temp, np.float32)
    n = x.shape[0]

    a = _bern_poly_coefs(temp)

    if x.shape == (N_NODES, FEATURES) and W1.shape == (FEATURES, NHID):
        h = None
        for attempt in range(2):
            try:
                h, _ = _mlp_trn(x, W1, b1, W2, b2)
                break
            except Exception as e:  # infrastructure failure only
                print(f"WARNING: TRN MLP attempt {attempt} failed "
                      f"({type(e).__name__}: {e})")
        if h is None:  # stay correct even if the device is wedged
            print("WARNING: falling back to numpy MLP")
            h = _mlp_numpy(x, W1, b1, W2, b2)
    else:
        h = _mlp_numpy(x, W1, b1, W2, b2)

    deg = 0
    for m in range(len(a) - 1, 0, -1):
        if a[m] != 0.0:
            deg = m
            break

    if deg == 0:
        out = h if a[0] == 1.0 else a[0] * h
        return np.ascontiguousarray(out.astype(np.float32))

    # General path (temp != initialized ones): Horner with deg(p) sparse
    # matvecs. Unreachable for the shipped problem instance.
    src = np.asarray(edge_index[0], np.int64)
    dst = np.asarray(edge_index[1], np.int64)
    deg_out = np.bincount(src, minlength=n).astype(np.float32)
    dinv = np.where(deg_out > 0, 1.0 / np.sqrt(np.maximum(deg_out, 1.0)), 0.0).astype(
        np.float32
    )
    w_edge = (dinv[src] * dinv[dst]).astype(np.float32)

    try:
        from scipy.sparse import coo_matrix

        A = coo_matrix((w_edge, (dst, src)), shape=(n, n)).tocsr()
        anorm = lambda z: (A @ z).astype(np.float32)
    except ImportError:
        def anorm(z):
            out = np.zeros_like(z)
            np.add.at(out, dst, w_edge[:, None] * z[src])
            return out

    z = (a[deg] * h).astype(np.float32)
    for m in range(deg - 1, -1, -1):
        z = (anorm(z) + a[m] * h).astype(np.float32)
    return np.ascontiguousarray(z.astype(np.float32))
